# revision 6
# baseline (speedup 1.0000x reference)
"""Trainium2 Bass kernel for nn_Critic (MLP preamble + 127-step LSTM + complex head).

Sharding: pure data-parallel over batch. B=256 -> 8 cores x 32. All weights
replicated; no collectives. Each core returns its [32, 512] output slice and
the host concatenates.

On-chip layout is "transposed" (feature-on-partition) end to end:
    h^T, c^T : [128 (hid%128), 4 (hid//128), 32 (batch)]
    z^T      : per-gate PSUM banks [128 (gate%128), 4 (gate//128), 32 (batch)]
so elementwise ops use all 128 lanes and h^T feeds the next step's matmul
(rhs) without any per-step transpose. All matmuls are bf16 inputs with fp32
PSUM accumulation (measured rel-L2 vs fp32 reference ~4e-3).

x@Wx + b_lstm for all 127 steps is hoisted out of the scan and computed as
one big parallel matmul into SBUF (bf16, ~127KB/partition), pipelined in
16-step chunks so transpose -> precompute -> recurrence overlap. The
recurrence keeps each LSTM gate in its own PSUM bank so its activation can
be evacuated while the Tensor engine is still producing the later gates;
only (z_o + X_o) -> sigmoid -> h=o*tanh(c) remains on the per-step tail.
"""

import numpy as np

import concourse.bass as bass  # noqa: F401
import concourse.mybir as mybir
from concourse import bacc, tile
from concourse.bass_utils import run_bass_kernel_spmd

dt = mybir.dt
AF = mybir.ActivationFunctionType

B = 32          # batch per core
NCORES = 8
T = 127         # scan steps (63 history + 64 action)
G = 2048        # gate dim
NM = G // 128   # 16 gate tiles
KH = 4          # hidden chunks of 128
KX = 2          # input chunks of 128
TC = 16         # time steps per pipeline chunk
# gate-tile order inside PSUM/XT blocks: [f, i, g, o] (original m-tile ids)
MPOS = [4, 5, 6, 7, 0, 1, 2, 3, 8, 9, 10, 11, 12, 13, 14, 15]

PER_BATCH = ("motion_state", "robot_state", "osc_state", "action", "osc", "history")
SHAPES = {
    "motion_state": (B, 128), "robot_state": (B, 64), "osc_state": (B, 128),
    "action": (B, 64, 256), "osc": (B, 128), "history": (B, 64, 256),
    "W_ros": (128, 256), "b_ros": (256,), "W_ios": (128, 256), "b_ios": (256,),
    "W_cos": (512, 256), "b_cos": (256,), "W_ms": (128, 256), "b_ms": (256,),
    "W_rs": (64, 256), "b_rs": (256,), "W_c": (768, 512), "b_c": (512,),
    "Wx": (256, 2048), "Wh": (512, 2048), "b_lstm": (2048,),
    "Wr_out": (256, 256), "Wi_out": (256, 256), "br_out": (256,), "bi_out": (256,),
}


def _bias_t(nc, pool, src_ap, n):
    """DMA a [n*128] bias vector into a [128, n] tile (partition = dim%128)."""
    t = pool.tile([128, n], dt.float32, name=f"bias_{src_ap.tensor.name}")
    nc.sync.dma_start(t[:, :], src_ap.rearrange("(j p) -> p j", p=128))
    return t


def build(tc, outs, ins, n_steps=T):
    nc = tc.nc
    f32, bf16 = dt.float32, dt.bfloat16
    Sig, Tanh = AF.Sigmoid, AF.Tanh
    nchunks = (n_steps + TC - 1) // TC

    import contextlib
    ctx = contextlib.ExitStack()
    with ctx:
        # ---------------- pools ----------------
        const = ctx.enter_context(tc.tile_pool(name="const", bufs=1))
        rec = ctx.enter_context(tc.tile_pool(name="rec", bufs=2))
        ptr = ctx.enter_context(tc.tile_pool(name="ptr", bufs=2, space="PSUM"))

        # identity via iota(j - p) == 0 — avoids affine_select's register
        # fill, which walrus fails to allocate on this compile path
        ident = const.tile([128, 128], f32, name="ident")
        iota_t = const.tile([128, 128], dt.int32, name="iota_t")
        nc.gpsimd.iota(iota_t[:, :], pattern=[[1, 128]], base=0,
                       channel_multiplier=-1)
        nc.vector.tensor_scalar(ident[:, :], iota_t[:, :], 0, None,
                                mybir.AluOpType.is_equal)

        # persistent weights / state
        Wh_bf = const.tile([128, KH, G], bf16, name="Wh_bf")
        Wr_bf = const.tile([128, 2, 256], bf16, name="Wr_bf")
        Wi_bf = const.tile([128, 2, 256], bf16, name="Wi_bf")
        Wineg_bf = const.tile([128, 2, 256], bf16, name="Wineg_bf")
        # XT split into time chunks so precompute -> recurrence pipelines
        XTc = [const.tile([128, NM, min(TC, n_steps - j * TC), B], bf16,
                          name=f"XT{j}") for j in range(nchunks)]
        c_st = const.tile([128, KH, B], f32, name="c_st")

        b_lstm_t = _bias_t(nc, const, ins["b_lstm"], NM)
        b_ros_t = _bias_t(nc, const, ins["b_ros"], 2)
        b_ios_t = _bias_t(nc, const, ins["b_ios"], 2)
        b_cos_t = _bias_t(nc, const, ins["b_cos"], 2)
        b_ms_t = _bias_t(nc, const, ins["b_ms"], 2)
        b_rs_t = _bias_t(nc, const, ins["b_rs"], 2)
        b_c_t = _bias_t(nc, const, ins["b_c"], 4)
        br_t = _bias_t(nc, const, ins["br_out"], 2)
        bi_t = _bias_t(nc, const, ins["bi_out"], 2)

        hT = None
        with tc.tile_pool(name="psetup", bufs=4, space="PSUM") as psetup:
            # ========= phase D: preamble MLP -> h0 = c0 = state^T (early) ===
            with tc.tile_pool(name="dpool", bufs=1) as dpool, \
                 tc.tile_pool(name="dload", bufs=2) as dload:

                def _load_w(name, kparts, n):
                    wt = dpool.tile([128, kparts, n], bf16, name=f"{name}_bf")
                    for k in range(kparts):
                        wtmp = dload.tile([128, 512], f32, tag="dl")
                        nc.sync.dma_start(wtmp[:, 0:n],
                                          ins[name][k * 128:(k + 1) * 128, :])
                        nc.any.tensor_copy(wt[:, k, :], wtmp[:, 0:n])
                    return wt

                Wros_bf = _load_w("W_ros", 1, 256)
                Wios_bf = _load_w("W_ios", 1, 256)
                Wms_bf = _load_w("W_ms", 1, 256)
                Wcos_bf = _load_w("W_cos", 4, 256)
                Wc_bf = _load_w("W_c", 6, 512)
                Wrs_bf = dpool.tile([128, 1, 256], bf16, name="W_rs_bf")
                wtmp = dload.tile([128, 512], f32, tag="dl")
                nc.sync.dma_start(wtmp[0:64, 0:256], ins["W_rs"][:, :])
                nc.any.tensor_copy(Wrs_bf[0:64, 0, :], wtmp[0:64, 0:256])

                def _tr_in(name, rows):
                    st = dload.tile([128, 512], f32, tag="dl")
                    nc.sync.dma_start(st[0:B, 0:rows], ins[name][:, :])
                    pt = ptr.tile([128, 128], f32, tag="tr")
                    nc.tensor.transpose(pt[0:rows, 0:B], st[0:B, 0:rows],
                                        ident[0:B, 0:B])
                    return pt

                p_mo = _tr_in("motion_state", 128)
                moT = dpool.tile([128, B], bf16, name="moT")
                nc.any.tensor_copy(moT[:, :], p_mo[:, 0:B])

                p_ro = _tr_in("robot_state", 64)
                roT = dpool.tile([128, B], bf16, name="roT")
                nc.any.tensor_copy(roT[0:64, :], p_ro[0:64, 0:B])

                reT = dpool.tile([128, B], bf16, name="reT")
                imT = dpool.tile([128, B], bf16, name="imT")
                p_os = _tr_in("osc_state", 128)
                nc.any.tensor_copy(reT[0:64, :], p_os[0:64, 0:B])
                nc.any.tensor_copy(imT[0:64, :], p_os[64:128, 0:B])
                p_oc = _tr_in("osc", 128)
                nc.any.tensor_copy(reT[64:128, :], p_oc[0:64, 0:B])
                nc.any.tensor_copy(imT[64:128, :], p_oc[64:128, 0:B])

                # stage 1: real_o / imag_o
                P1 = psetup.tile([128, 512], f32, tag="s")
                for m in range(2):
                    nc.tensor.matmul(P1[:, m * B:(m + 1) * B],
                                     lhsT=Wros_bf[:, 0, m * 128:(m + 1) * 128],
                                     rhs=reT[:, :], start=True, stop=True)
                for m in range(2):
                    nc.tensor.matmul(P1[:, (2 + m) * B:(3 + m) * B],
                                     lhsT=Wios_bf[:, 0, m * 128:(m + 1) * 128],
                                     rhs=imT[:, :], start=True, stop=True)
                ro_bf = dpool.tile([128, 2, B], bf16, name="ro_bf")
                io_bf = dpool.tile([128, 2, B], bf16, name="io_bf")
                for m in range(2):
                    nc.scalar.activation(ro_bf[:, m, :], P1[:, m * B:(m + 1) * B],
                                         Tanh, bias=b_ros_t[:, m:m + 1])
                    nc.scalar.activation(io_bf[:, m, :],
                                         P1[:, (2 + m) * B:(3 + m) * B],
                                         Tanh, bias=b_ios_t[:, m:m + 1])

                # stage 2: ms, rs, osc_s
                P2 = psetup.tile([128, 512], f32, tag="s")
                for m in range(2):
                    nc.tensor.matmul(P2[:, m * B:(m + 1) * B],
                                     lhsT=Wms_bf[:, 0, m * 128:(m + 1) * 128],
                                     rhs=moT[:, :], start=True, stop=True)
                for m in range(2):
                    nc.tensor.matmul(P2[:, (2 + m) * B:(3 + m) * B],
                                     lhsT=Wrs_bf[0:64, 0, m * 128:(m + 1) * 128],
                                     rhs=roT[0:64, :], start=True, stop=True)
                cos_chunks = [ro_bf[:, 0, :], ro_bf[:, 1, :],
                              io_bf[:, 0, :], io_bf[:, 1, :]]
                for m in range(2):
                    for k in range(4):
                        nc.tensor.matmul(P2[:, (4 + m) * B:(5 + m) * B],
                                         lhsT=Wcos_bf[:, k, m * 128:(m + 1) * 128],
                                         rhs=cos_chunks[k],
                                         start=(k == 0), stop=(k == 3))
                ms_bf = dpool.tile([128, 2, B], bf16, name="ms_bf")
                rs_bf = dpool.tile([128, 2, B], bf16, name="rs_bf")
                os_bf = dpool.tile([128, 2, B], bf16, name="os_bf")
                for m in range(2):
                    nc.scalar.activation(ms_bf[:, m, :], P2[:, m * B:(m + 1) * B],
                                         Tanh, bias=b_ms_t[:, m:m + 1])
                    nc.scalar.activation(rs_bf[:, m, :],
                                         P2[:, (2 + m) * B:(3 + m) * B],
                                         Tanh, bias=b_rs_t[:, m:m + 1])
                    nc.scalar.activation(os_bf[:, m, :],
                                         P2[:, (4 + m) * B:(5 + m) * B],
                                         Tanh, bias=b_cos_t[:, m:m + 1])

                # stage 3: state = tanh([ms rs osc_s] @ W_c + b_c) -> h0 = c0
                P3 = psetup.tile([128, 512], f32, tag="s")
                st_chunks = [ms_bf[:, 0, :], ms_bf[:, 1, :], rs_bf[:, 0, :],
                             rs_bf[:, 1, :], os_bf[:, 0, :], os_bf[:, 1, :]]
                for m in range(KH):
                    for k in range(6):
                        nc.tensor.matmul(P3[:, m * B:(m + 1) * B],
                                         lhsT=Wc_bf[:, k, m * 128:(m + 1) * 128],
                                         rhs=st_chunks[k],
                                         start=(k == 0), stop=(k == 5))
                hT = rec.tile([128, KH, B], bf16, tag="h")
                for m in range(KH):
                    nc.scalar.activation(c_st[:, m, :], P3[:, m * B:(m + 1) * B],
                                         Tanh, bias=b_c_t[:, m:m + 1])
                nc.vector.tensor_copy(hT[:, :, :], c_st[:, :, :])

            # ======= phases A+B+C: weights, seq transpose, X precompute =====
            with tc.tile_pool(name="pre", bufs=1) as pre, \
                 tc.tile_pool(name="wload", bufs=3) as wload, \
                 tc.tile_pool(name="seqload", bufs=4) as seqload:

                # ---- A: big weights -> bf16 sbuf
                Wx_bf = pre.tile([128, KX, G], bf16, name="Wx_bf")
                for k in range(KH):
                    for h2 in range(2):
                        wtmp = wload.tile([128, 1024], f32, tag="wl")
                        nc.sync.dma_start(
                            wtmp[:, :], ins["Wh"][k * 128:(k + 1) * 128,
                                                  h2 * 1024:(h2 + 1) * 1024])
                        nc.any.tensor_copy(
                            Wh_bf[:, k, h2 * 1024:(h2 + 1) * 1024], wtmp[:, :])
                for k in range(KX):
                    for h2 in range(2):
                        wtmp = wload.tile([128, 1024], f32, tag="wl")
                        nc.sync.dma_start(
                            wtmp[:, :], ins["Wx"][k * 128:(k + 1) * 128,
                                                  h2 * 1024:(h2 + 1) * 1024])
                        nc.any.tensor_copy(
                            Wx_bf[:, k, h2 * 1024:(h2 + 1) * 1024], wtmp[:, :])
                for k in range(2):
                    wtmp = wload.tile([128, 1024], f32, tag="wl")
                    nc.sync.dma_start(wtmp[:, 0:256],
                                      ins["Wr_out"][k * 128:(k + 1) * 128, :])
                    nc.any.tensor_copy(Wr_bf[:, k, :], wtmp[:, 0:256])
                    wtmp = wload.tile([128, 1024], f32, tag="wl")
                    nc.sync.dma_start(wtmp[:, 0:256],
                                      ins["Wi_out"][k * 128:(k + 1) * 128, :])
                    nc.any.tensor_copy(Wi_bf[:, k, :], wtmp[:, 0:256])
                    nc.scalar.mul(Wineg_bf[:, k, :], wtmp[:, 0:256], -1.0)

                # ---- B+C interleaved per 16-step chunk: transpose seq ->
                # xT chunk (bf16, feat-on-partition), then X = Wx^T x + b
                hist, act = ins["history"], ins["action"]
                xTc = [pre.tile([128, KX, min(TC, n_steps - j * TC) * B], bf16,
                                name=f"xT{j}") for j in range(nchunks)]
                for ch in range(nchunks):
                    tcnt = min(TC, n_steps - ch * TC)
                    cols = tcnt * B
                    # B: transpose 4 packs of 4 steps
                    for pk in range((tcnt + 3) // 4):
                        t0 = ch * TC + 4 * pk
                        nt = min(4, n_steps - t0)
                        st = seqload.tile([128, 256], f32, tag="seq")
                        tcur = t0
                        while tcur < t0 + nt:
                            if tcur < 63:
                                cnt = min(63 - tcur, t0 + nt - tcur)
                                src = hist[:, tcur:tcur + cnt, :]
                            else:
                                cnt = t0 + nt - tcur
                                src = act[:, tcur - 63:tcur - 63 + cnt, :]
                            row0 = (tcur - t0) * B
                            nc.sync.dma_start(
                                st[row0:row0 + cnt * B, :],
                                src.rearrange("b t f -> t b f"))
                            tcur += cnt
                        for fc in range(KX):
                            pt = ptr.tile([128, 128], f32, tag="tr")
                            nc.tensor.transpose(
                                pt[:, 0:nt * B],
                                st[0:nt * B, fc * 128:(fc + 1) * 128],
                                ident[0:nt * B, 0:nt * B])
                            nc.any.tensor_copy(
                                xTc[ch][:, fc, 4 * pk * B:(4 * pk + nt) * B],
                                pt[:, 0:nt * B])
                    # C: X[t] for this chunk, all 16 gate tiles
                    for pos in range(NM):
                        m = MPOS[pos]
                        P = psetup.tile([128, 512], f32, tag="s")
                        for k in range(KX):
                            nc.tensor.matmul(
                                P[:, 0:cols],
                                lhsT=Wx_bf[:, k, m * 128:(m + 1) * 128],
                                rhs=xTc[ch][:, k, 0:cols],
                                start=(k == 0), stop=(k == KX - 1))
                        XTm = XTc[ch][:, pos, :, :].rearrange("p t b -> p (t b)")
                        if pos % 2 == 0:
                            nc.scalar.activation(XTm[:, 0:cols], P[:, 0:cols],
                                                 AF.Identity,
                                                 bias=b_lstm_t[:, m:m + 1])
                        else:
                            nc.vector.tensor_scalar_add(XTm[:, 0:cols],
                                                        P[:, 0:cols],
                                                        b_lstm_t[:, m:m + 1])

        # ============ phase E: LSTM recurrence over n_steps ============
        # per-gate PSUM banks so activations evacuate while PE still works
        pz = ctx.enter_context(tc.tile_pool(name="pz", bufs=1, space="PSUM"))
        for t in range(n_steps):
            ch, tl = t // TC, t % TC
            Zf = pz.tile([128, KH, B], f32, tag="zf")
            Zi = pz.tile([128, KH, B], f32, tag="zi")
            Zg = pz.tile([128, KH, B], f32, tag="zg")
            Zo = pz.tile([128, KH, B], f32, tag="zo")
            Zs = [Zf, Zi, Zg, Zo]
            gates = rec.tile([128, NM, B], f32, tag="gates")
            # gates layout follows MPOS order: [f, i, g, o] blocks of 4
            gf, gi = gates[:, 0:4, :], gates[:, 4:8, :]
            gg, go = gates[:, 8:12, :], gates[:, 12:16, :]
            zsb = rec.tile([128, NM, B], f32, tag="zsb")
            tmp = rec.tile([128, KH, B], f32, tag="tmp")
            tanh_c = rec.tile([128, KH, B], f32, tag="tanhc")
            hT_n = rec.tile([128, KH, B], bf16, tag="h")

            for gidx in range(4):           # f, i, g, o gate groups
                Z = Zs[gidx]
                for blk in range(4):
                    m = MPOS[gidx * 4 + blk]
                    for k in range(KH):
                        nc.tensor.matmul(Z[:, blk, :],
                                         lhsT=Wh_bf[:, k, m * 128:(m + 1) * 128],
                                         rhs=hT[:, k, :],
                                         start=(k == 0), stop=(k == KH - 1))
                # DVE: z = Z + X[t] for this gate; ACT consumes right after
                sl = slice(gidx * 4, gidx * 4 + 4)
                nc.vector.tensor_add(zsb[:, sl, :], Z[:, :, :],
                                     XTc[ch][:, sl, tl, :])
                if gidx == 0:
                    nc.scalar.activation(gf[:, :, :], zsb[:, sl, :], Sig)
                elif gidx == 1:
                    nc.scalar.activation(gi[:, :, :], zsb[:, sl, :], Sig)
                    # c = f*c can start once sig_f done
                    nc.vector.tensor_mul(c_st[:, :, :], gf[:, :, :],
                                         c_st[:, :, :])
                elif gidx == 2:
                    nc.scalar.activation(gg[:, :, :], zsb[:, sl, :], Tanh)
                    nc.vector.tensor_mul(tmp[:, :, :], gi[:, :, :], gg[:, :, :])
                    nc.vector.tensor_add(c_st[:, :, :], c_st[:, :, :],
                                         tmp[:, :, :])
                    # tanh(c) before sig_o in the ACT queue: it only waits on
                    # the DVE chain, which finishes during the o matmuls
                    nc.scalar.activation(tanh_c[:, :, :], c_st[:, :, :], Tanh)
                else:
                    nc.scalar.activation(go[:, :, :], zsb[:, sl, :], Sig)
                    nc.vector.tensor_mul(hT_n[:, :, :], go[:, :, :],
                                         tanh_c[:, :, :])
            hT = hT_n

        # ============ phase F: complex dense head + output transpose ========
        P4 = pz.tile([128, KH, B], f32, tag="zf")
        for m in range(2):
            for k in range(2):
                nc.tensor.matmul(P4[:, m, :],
                                 lhsT=Wr_bf[:, k, m * 128:(m + 1) * 128],
                                 rhs=hT[:, k, :], start=(k == 0), stop=False)
            for k in range(2):
                nc.tensor.matmul(P4[:, m, :],
                                 lhsT=Wineg_bf[:, k, m * 128:(m + 1) * 128],
                                 rhs=hT[:, 2 + k, :], start=False, stop=(k == 1))
            for k in range(2):
                nc.tensor.matmul(P4[:, 2 + m, :],
                                 lhsT=Wi_bf[:, k, m * 128:(m + 1) * 128],
                                 rhs=hT[:, k, :], start=(k == 0), stop=False)
            for k in range(2):
                nc.tensor.matmul(P4[:, 2 + m, :],
                                 lhsT=Wr_bf[:, k, m * 128:(m + 1) * 128],
                                 rhs=hT[:, 2 + k, :], start=False, stop=(k == 1))
        outT = rec.tile([128, 4, B], f32, tag="outT")
        for m in range(2):
            nc.scalar.activation(outT[:, m, :], P4[:, m, :], Tanh,
                                 bias=br_t[:, m:m + 1])
            nc.scalar.activation(outT[:, 2 + m, :], P4[:, 2 + m, :], Tanh,
                                 bias=bi_t[:, m:m + 1])
        out_sb = rec.tile([128, 512], f32, tag="out_sb")
        for j in range(4):
            pt = ptr.tile([128, 128], f32, tag="tr")
            nc.tensor.transpose(pt[0:B, 0:128], outT[:, j, :], ident[:, :])
            nc.any.tensor_copy(out_sb[0:B, j * 128:(j + 1) * 128], pt[0:B, 0:128])
        nc.sync.dma_start(outs["out"][:, :], out_sb[0:B, :])


_cached_nc = None


def _get_program():
    global _cached_nc
    if _cached_nc is None:
        nc = bacc.Bacc("TRN2", target_bir_lowering=False, debug=False)
        ins = {}
        for name, shape in SHAPES.items():
            ins[name] = nc.dram_tensor(name, list(shape), dt.float32,
                                       kind="ExternalInput")[...]
        out = nc.dram_tensor("out", [B, 512], dt.float32, kind="ExternalOutput")
        with tile.TileContext(nc) as tc:
            build(tc, {"out": out[...]}, ins)
        nc.finalize()  # bacc legalization (wait splitting, reg alloc, DCE)
        _cached_nc = nc
    return _cached_nc


def kernel(**inputs):
    nc = _get_program()
    in_maps = []
    for i in range(NCORES):
        m = {}
        for name in SHAPES:
            arr = np.ascontiguousarray(inputs[name], dtype=np.float32)
            if name in PER_BATCH:
                arr = np.ascontiguousarray(arr[i * B:(i + 1) * B])
            m[name] = arr
        in_maps.append(m)
    res = run_bass_kernel_spmd(nc, in_maps, list(range(NCORES)))
    return np.concatenate([res.results[i]["out"] for i in range(NCORES)], axis=0)


if __name__ == "__main__":
    import reference  # noqa: F401  (only for a local smoke run)
    inp = {k: np.asarray(v) for k, v in reference.setup_inputs().items()}
    out = kernel(**inp)
    print(out.shape, out.dtype)


# revision 12
# speedup vs baseline: 1.2503x; 1.2503x over previous
"""Trainium2 Bass kernel for nn_Critic (MLP preamble + 127-step LSTM + complex head).

Sharding: pure data-parallel over batch. B=256 -> 8 cores x 32. All weights
replicated; no collectives. Each core returns its [32, 512] output slice and
the host concatenates.

On-chip layout is "transposed" (feature-on-partition) end to end:
    h^T, c^T : [128 (hid%128), 4 (hid//128), 32 (batch)]
    z^T      : per-gate PSUM banks [128 (gate%128), 4 (gate//128), 32 (batch)]
so elementwise ops use all 128 lanes and h^T feeds the next step's matmul
(rhs) without any per-step transpose. All matmuls are bf16 inputs with fp32
PSUM accumulation (measured rel-L2 vs fp32 reference ~4e-3).

x@Wx + b_lstm for all 127 steps is hoisted out of the scan and computed as
one big parallel matmul into SBUF (bf16, ~127KB/partition), pipelined in
16-step chunks so transpose -> precompute -> recurrence overlap. The
recurrence keeps each LSTM gate in its own PSUM bank so its activation can
be evacuated while the Tensor engine is still producing the later gates;
only (z_o + X_o) -> sigmoid -> h=o*tanh(c) remains on the per-step tail.
"""

import numpy as np

import concourse.bass as bass  # noqa: F401
import concourse.mybir as mybir
from concourse import bacc, tile
from concourse.bass_utils import run_bass_kernel_spmd

dt = mybir.dt
AF = mybir.ActivationFunctionType

B = 32          # batch per core
NCORES = 8
T = 127         # scan steps (63 history + 64 action)
G = 2048        # gate dim
NM = G // 128   # 16 gate tiles
KH = 4          # hidden chunks of 128
KX = 2          # input chunks of 128
TC = 16         # time steps per pipeline chunk
# gate-tile order inside PSUM/XT blocks: [f, i, g, o] (original m-tile ids)
MPOS = [4, 5, 6, 7, 0, 1, 2, 3, 8, 9, 10, 11, 12, 13, 14, 15]

PER_BATCH = ("motion_state", "robot_state", "osc_state", "action", "osc", "history")
SHAPES = {
    "motion_state": (B, 128), "robot_state": (B, 64), "osc_state": (B, 128),
    "action": (B, 64, 256), "osc": (B, 128), "history": (B, 64, 256),
    "W_ros": (128, 256), "b_ros": (256,), "W_ios": (128, 256), "b_ios": (256,),
    "W_cos": (512, 256), "b_cos": (256,), "W_ms": (128, 256), "b_ms": (256,),
    "W_rs": (64, 256), "b_rs": (256,), "W_c": (768, 512), "b_c": (512,),
    "Wx": (256, 2048), "Wh": (512, 2048), "b_lstm": (2048,),
    "Wr_out": (256, 256), "Wi_out": (256, 256), "br_out": (256,), "bi_out": (256,),
}


def _bias_t(nc, pool, src_ap, n):
    """DMA a [n*128] bias vector into a [128, n] tile (partition = dim%128)."""
    t = pool.tile([128, n], dt.float32, name=f"bias_{src_ap.tensor.name}")
    nc.sync.dma_start(t[:, :], src_ap.rearrange("(j p) -> p j", p=128))
    return t


def build(tc, outs, ins, n_steps=T):
    nc = tc.nc
    f32, bf16 = dt.float32, dt.bfloat16
    Sig, Tanh = AF.Sigmoid, AF.Tanh
    nchunks = (n_steps + TC - 1) // TC

    import contextlib
    ctx = contextlib.ExitStack()
    with ctx:
        # ---------------- pools ----------------
        const = ctx.enter_context(tc.tile_pool(name="const", bufs=1))
        rec = ctx.enter_context(tc.tile_pool(name="rec", bufs=2))
        ptr = ctx.enter_context(tc.tile_pool(name="ptr", bufs=2, space="PSUM"))

        # identity via iota(j - p) == 0 — avoids affine_select's register
        # fill, which walrus fails to allocate on this compile path
        ident = const.tile([128, 128], f32, name="ident")
        iota_t = const.tile([128, 128], dt.int32, name="iota_t")
        nc.gpsimd.iota(iota_t[:, :], pattern=[[1, 128]], base=0,
                       channel_multiplier=-1)
        nc.vector.tensor_scalar(ident[:, :], iota_t[:, :], 0, None,
                                mybir.AluOpType.is_equal)
        ident_bf = const.tile([128, 128], bf16, name="ident_bf")
        nc.vector.tensor_copy(ident_bf[:, :], ident[:, :])

        # persistent weights / state
        Wh_bf = const.tile([128, KH, G], bf16, name="Wh_bf")
        Wr_bf = const.tile([128, 2, 256], bf16, name="Wr_bf")
        Wi_bf = const.tile([128, 2, 256], bf16, name="Wi_bf")
        Wineg_bf = const.tile([128, 2, 256], bf16, name="Wineg_bf")
        # XT split into time chunks so precompute -> recurrence pipelines
        XTc = [const.tile([128, NM, min(TC, n_steps - j * TC), B], bf16,
                          name=f"XT{j}") for j in range(nchunks)]
        c_st = const.tile([128, KH, B], f32, name="c_st")

        b_lstm_t = _bias_t(nc, const, ins["b_lstm"], NM)
        b_ros_t = _bias_t(nc, const, ins["b_ros"], 2)
        b_ios_t = _bias_t(nc, const, ins["b_ios"], 2)
        b_cos_t = _bias_t(nc, const, ins["b_cos"], 2)
        b_ms_t = _bias_t(nc, const, ins["b_ms"], 2)
        b_rs_t = _bias_t(nc, const, ins["b_rs"], 2)
        b_c_t = _bias_t(nc, const, ins["b_c"], 4)
        br_t = _bias_t(nc, const, ins["br_out"], 2)
        bi_t = _bias_t(nc, const, ins["bi_out"], 2)

        hT = None
        with tc.tile_pool(name="psetup", bufs=4, space="PSUM") as psetup:
            # ========= phase D: preamble MLP -> h0 = c0 = state^T (early) ===
            with tc.tile_pool(name="dpool", bufs=1) as dpool, \
                 tc.tile_pool(name="dload", bufs=2) as dload:

                def _load_w(name, kparts, n):
                    wt = dpool.tile([128, kparts, n], bf16, name=f"{name}_bf")
                    for k in range(kparts):
                        wtmp = dload.tile([128, 512], f32, tag="dl")
                        nc.sync.dma_start(wtmp[:, 0:n],
                                          ins[name][k * 128:(k + 1) * 128, :])
                        nc.any.tensor_copy(wt[:, k, :], wtmp[:, 0:n])
                    return wt

                Wros_bf = _load_w("W_ros", 1, 256)
                Wios_bf = _load_w("W_ios", 1, 256)
                Wms_bf = _load_w("W_ms", 1, 256)
                Wcos_bf = _load_w("W_cos", 4, 256)
                Wc_bf = _load_w("W_c", 6, 512)
                Wrs_bf = dpool.tile([128, 1, 256], bf16, name="W_rs_bf")
                wtmp = dload.tile([128, 512], f32, tag="dl")
                nc.sync.dma_start(wtmp[0:64, 0:256], ins["W_rs"][:, :])
                nc.any.tensor_copy(Wrs_bf[0:64, 0, :], wtmp[0:64, 0:256])

                def _tr_in(name, rows):
                    st = dload.tile([128, 512], f32, tag="dl")
                    nc.sync.dma_start(st[0:B, 0:rows], ins[name][:, :])
                    pt = ptr.tile([128, 128], f32, tag="tr")
                    nc.tensor.transpose(pt[0:rows, 0:B], st[0:B, 0:rows],
                                        ident[0:B, 0:B])
                    return pt

                p_mo = _tr_in("motion_state", 128)
                moT = dpool.tile([128, B], bf16, name="moT")
                nc.any.tensor_copy(moT[:, :], p_mo[:, 0:B])

                p_ro = _tr_in("robot_state", 64)
                roT = dpool.tile([128, B], bf16, name="roT")
                nc.any.tensor_copy(roT[0:64, :], p_ro[0:64, 0:B])

                reT = dpool.tile([128, B], bf16, name="reT")
                imT = dpool.tile([128, B], bf16, name="imT")
                p_os = _tr_in("osc_state", 128)
                nc.any.tensor_copy(reT[0:64, :], p_os[0:64, 0:B])
                nc.any.tensor_copy(imT[0:64, :], p_os[64:128, 0:B])
                p_oc = _tr_in("osc", 128)
                nc.any.tensor_copy(reT[64:128, :], p_oc[0:64, 0:B])
                nc.any.tensor_copy(imT[64:128, :], p_oc[64:128, 0:B])

                # stage 1: real_o / imag_o
                P1 = psetup.tile([128, 512], f32, tag="s")
                for m in range(2):
                    nc.tensor.matmul(P1[:, m * B:(m + 1) * B],
                                     lhsT=Wros_bf[:, 0, m * 128:(m + 1) * 128],
                                     rhs=reT[:, :], start=True, stop=True)
                for m in range(2):
                    nc.tensor.matmul(P1[:, (2 + m) * B:(3 + m) * B],
                                     lhsT=Wios_bf[:, 0, m * 128:(m + 1) * 128],
                                     rhs=imT[:, :], start=True, stop=True)
                ro_bf = dpool.tile([128, 2, B], bf16, name="ro_bf")
                io_bf = dpool.tile([128, 2, B], bf16, name="io_bf")
                for m in range(2):
                    nc.scalar.activation(ro_bf[:, m, :], P1[:, m * B:(m + 1) * B],
                                         Tanh, bias=b_ros_t[:, m:m + 1])
                    nc.scalar.activation(io_bf[:, m, :],
                                         P1[:, (2 + m) * B:(3 + m) * B],
                                         Tanh, bias=b_ios_t[:, m:m + 1])

                # stage 2: ms, rs, osc_s
                P2 = psetup.tile([128, 512], f32, tag="s")
                for m in range(2):
                    nc.tensor.matmul(P2[:, m * B:(m + 1) * B],
                                     lhsT=Wms_bf[:, 0, m * 128:(m + 1) * 128],
                                     rhs=moT[:, :], start=True, stop=True)
                for m in range(2):
                    nc.tensor.matmul(P2[:, (2 + m) * B:(3 + m) * B],
                                     lhsT=Wrs_bf[0:64, 0, m * 128:(m + 1) * 128],
                                     rhs=roT[0:64, :], start=True, stop=True)
                cos_chunks = [ro_bf[:, 0, :], ro_bf[:, 1, :],
                              io_bf[:, 0, :], io_bf[:, 1, :]]
                for m in range(2):
                    for k in range(4):
                        nc.tensor.matmul(P2[:, (4 + m) * B:(5 + m) * B],
                                         lhsT=Wcos_bf[:, k, m * 128:(m + 1) * 128],
                                         rhs=cos_chunks[k],
                                         start=(k == 0), stop=(k == 3))
                ms_bf = dpool.tile([128, 2, B], bf16, name="ms_bf")
                rs_bf = dpool.tile([128, 2, B], bf16, name="rs_bf")
                os_bf = dpool.tile([128, 2, B], bf16, name="os_bf")
                for m in range(2):
                    nc.scalar.activation(ms_bf[:, m, :], P2[:, m * B:(m + 1) * B],
                                         Tanh, bias=b_ms_t[:, m:m + 1])
                    nc.scalar.activation(rs_bf[:, m, :],
                                         P2[:, (2 + m) * B:(3 + m) * B],
                                         Tanh, bias=b_rs_t[:, m:m + 1])
                    nc.scalar.activation(os_bf[:, m, :],
                                         P2[:, (4 + m) * B:(5 + m) * B],
                                         Tanh, bias=b_cos_t[:, m:m + 1])

                # stage 3: state = tanh([ms rs osc_s] @ W_c + b_c) -> h0 = c0
                P3 = psetup.tile([128, 512], f32, tag="s")
                st_chunks = [ms_bf[:, 0, :], ms_bf[:, 1, :], rs_bf[:, 0, :],
                             rs_bf[:, 1, :], os_bf[:, 0, :], os_bf[:, 1, :]]
                for m in range(KH):
                    for k in range(6):
                        nc.tensor.matmul(P3[:, m * B:(m + 1) * B],
                                         lhsT=Wc_bf[:, k, m * 128:(m + 1) * 128],
                                         rhs=st_chunks[k],
                                         start=(k == 0), stop=(k == 5))
                hTb = [rec.tile([128, B], bf16, tag=f"h{k}", bufs=3, name=f"h0_{k}")
                       for k in range(KH)]
                for m in range(KH):
                    nc.scalar.activation(c_st[:, m, :], P3[:, m * B:(m + 1) * B],
                                         Tanh, bias=b_c_t[:, m:m + 1])
                for k in range(KH):
                    nc.vector.tensor_copy(hTb[k][:, :], c_st[:, k, :])
                hT = hTb

            # ======= phases A+B+C: weights, seq transpose, X precompute =====
            with tc.tile_pool(name="pre", bufs=1) as pre, \
                 tc.tile_pool(name="wload", bufs=3) as wload, \
                 tc.tile_pool(name="seqload", bufs=4) as seqload:

                # ---- A: big weights -> bf16 sbuf
                Wx_bf = pre.tile([128, KX, G], bf16, name="Wx_bf")
                for k in range(KH):
                    for h2 in range(2):
                        wtmp = wload.tile([128, 1024], f32, tag="wl")
                        nc.sync.dma_start(
                            wtmp[:, :], ins["Wh"][k * 128:(k + 1) * 128,
                                                  h2 * 1024:(h2 + 1) * 1024])
                        nc.any.tensor_copy(
                            Wh_bf[:, k, h2 * 1024:(h2 + 1) * 1024], wtmp[:, :])
                for k in range(KX):
                    for h2 in range(2):
                        wtmp = wload.tile([128, 1024], f32, tag="wl")
                        nc.sync.dma_start(
                            wtmp[:, :], ins["Wx"][k * 128:(k + 1) * 128,
                                                  h2 * 1024:(h2 + 1) * 1024])
                        nc.any.tensor_copy(
                            Wx_bf[:, k, h2 * 1024:(h2 + 1) * 1024], wtmp[:, :])
                for k in range(2):
                    wtmp = wload.tile([128, 1024], f32, tag="wl")
                    nc.sync.dma_start(wtmp[:, 0:256],
                                      ins["Wr_out"][k * 128:(k + 1) * 128, :])
                    nc.any.tensor_copy(Wr_bf[:, k, :], wtmp[:, 0:256])
                    wtmp = wload.tile([128, 1024], f32, tag="wl")
                    nc.sync.dma_start(wtmp[:, 0:256],
                                      ins["Wi_out"][k * 128:(k + 1) * 128, :])
                    nc.any.tensor_copy(Wi_bf[:, k, :], wtmp[:, 0:256])
                    nc.scalar.mul(Wineg_bf[:, k, :], wtmp[:, 0:256], -1.0)

                # ---- B+C interleaved per 16-step chunk: transpose seq ->
                # xT chunk (bf16, feat-on-partition), then X = Wx^T x + b
                hist, act = ins["history"], ins["action"]
                xTc = [pre.tile([128, KX, min(TC, n_steps - j * TC) * B], bf16,
                                name=f"xT{j}") for j in range(nchunks)]
                for ch in range(nchunks):
                    tcnt = min(TC, n_steps - ch * TC)
                    cols = tcnt * B
                    # B: transpose 4 packs of 4 steps
                    for pk in range((tcnt + 3) // 4):
                        t0 = ch * TC + 4 * pk
                        nt = min(4, n_steps - t0)
                        st = seqload.tile([128, 256], f32, tag="seq")
                        tcur = t0
                        while tcur < t0 + nt:
                            if tcur < 63:
                                cnt = min(63 - tcur, t0 + nt - tcur)
                                src = hist[:, tcur:tcur + cnt, :]
                            else:
                                cnt = t0 + nt - tcur
                                src = act[:, tcur - 63:tcur - 63 + cnt, :]
                            row0 = (tcur - t0) * B
                            nc.sync.dma_start(
                                st[row0:row0 + cnt * B, :],
                                src.rearrange("b t f -> t b f"))
                            tcur += cnt
                        for fc in range(KX):
                            pt = ptr.tile([128, 128], f32, tag="tr")
                            nc.tensor.transpose(
                                pt[:, 0:nt * B],
                                st[0:nt * B, fc * 128:(fc + 1) * 128],
                                ident[0:nt * B, 0:nt * B])
                            nc.any.tensor_copy(
                                xTc[ch][:, fc, 4 * pk * B:(4 * pk + nt) * B],
                                pt[:, 0:nt * B])
                    # C: X[t] for this chunk, all 16 gate tiles
                    for pos in range(NM):
                        m = MPOS[pos]
                        P = psetup.tile([128, 512], f32, tag="s")
                        for k in range(KX):
                            nc.tensor.matmul(
                                P[:, 0:cols],
                                lhsT=Wx_bf[:, k, m * 128:(m + 1) * 128],
                                rhs=xTc[ch][:, k, 0:cols],
                                start=(k == 0), stop=(k == KX - 1))
                        XTm = XTc[ch][:, pos, :, :].rearrange("p t b -> p (t b)")
                        if pos % 2 == 0:
                            nc.scalar.activation(XTm[:, 0:cols], P[:, 0:cols],
                                                 AF.Identity,
                                                 bias=b_lstm_t[:, m:m + 1])
                        else:
                            nc.vector.tensor_scalar_add(XTm[:, 0:cols],
                                                        P[:, 0:cols],
                                                        b_lstm_t[:, m:m + 1])

        # ============ phase E: LSTM recurrence over n_steps ============
        # per-gate PSUM banks so activations evacuate while PE still works
        pz = ctx.enter_context(tc.tile_pool(name="pz", bufs=1, space="PSUM"))
        for t in range(n_steps):
            ch, tl = t // TC, t % TC
            Zf = pz.tile([128, KH, B], f32, tag="zf")
            Zi = pz.tile([128, KH, B], f32, tag="zi")
            Zg = pz.tile([128, KH, B], f32, tag="zg")
            Zo = pz.tile([128, KH, B], f32, tag="zo")
            Zs = [Zf, Zi, Zg, Zo]
            gf = rec.tile([128, KH, B], f32, tag="gf")
            gi = rec.tile([128, KH, B], f32, tag="gi")
            gg = rec.tile([128, KH, B], f32, tag="gg")
            go = rec.tile([128, KH, B], f32, tag="go")
            tmp = rec.tile([128, KH, B], f32, tag="tmp")
            tanh_c = rec.tile([128, KH, B], f32, tag="tanhc")
            hTb_n = [rec.tile([128, B], bf16, tag=f"h{k}", bufs=3, name=f"h{t}_{k}")
                     for k in range(KH)]

            # X[t] injected into PSUM by the PE itself (identity stationary):
            # no h dependency, so these run during the previous step's tail,
            # and the activations can read the finished bank directly.
            for gidx in range(4):
                nc.tensor.matmul(Zs[gidx][:, :, :], lhsT=ident_bf[:, :],
                                 rhs=XTc[ch][:, gidx * 4:gidx * 4 + 4, tl, :],
                                 start=True, stop=False, skip_group_check=True)
            # k=0 level for all 16 gate tiles: only needs h block 0, which the
            # split h-mul below releases first
            for gidx in range(4):
                for blk in range(4):
                    m = MPOS[gidx * 4 + blk]
                    nc.tensor.matmul(Zs[gidx][:, blk, :],
                                     lhsT=Wh_bf[:, 0, m * 128:(m + 1) * 128],
                                     rhs=hT[0][:, :], start=False, stop=False,
                                     skip_group_check=True)
            # per-gate k=1..3 batches; each gate's bank completes early so its
            # activation overlaps the later gates' matmuls
            for gidx in range(4):
                Z = Zs[gidx]
                for blk in range(4):
                    m = MPOS[gidx * 4 + blk]
                    for k in range(1, KH):
                        nc.tensor.matmul(
                            Z[:, blk, :],
                            lhsT=Wh_bf[:, k, m * 128:(m + 1) * 128],
                            rhs=hT[k][:, :], start=False,
                            stop=(k == KH - 1 and blk == 3),
                            skip_group_check=True)
                if gidx == 0:
                    nc.scalar.activation(gf[:, :, :], Z[:, :, :], Sig)
                elif gidx == 1:
                    nc.scalar.activation(gi[:, :, :], Z[:, :, :], Sig)
                    nc.vector.tensor_mul(c_st[:, :, :], gf[:, :, :],
                                         c_st[:, :, :])
                elif gidx == 2:
                    nc.scalar.activation(gg[:, :, :], Z[:, :, :], Tanh)
                    nc.vector.tensor_mul(tmp[:, :, :], gi[:, :, :], gg[:, :, :])
                    nc.vector.tensor_add(c_st[:, :, :], c_st[:, :, :],
                                         tmp[:, :, :])
                    # tanh(c) before sig_o in the ACT queue: it only waits on
                    # the DVE chain, which finishes during the o matmuls
                    nc.scalar.activation(tanh_c[:, :, :], c_st[:, :, :], Tanh)
                else:
                    nc.scalar.activation(go[:, :, :], Z[:, :, :], Sig)
                    for k in range(KH):
                        nc.vector.tensor_mul(hTb_n[k][:, :], go[:, k, :],
                                             tanh_c[:, k, :])
            hT = hTb_n

        # ============ phase F: complex dense head + output transpose ========
        P4 = pz.tile([128, KH, B], f32, tag="zf")
        for m in range(2):
            for k in range(2):
                nc.tensor.matmul(P4[:, m, :],
                                 lhsT=Wr_bf[:, k, m * 128:(m + 1) * 128],
                                 rhs=hT[k][:, :], start=(k == 0), stop=False)
            for k in range(2):
                nc.tensor.matmul(P4[:, m, :],
                                 lhsT=Wineg_bf[:, k, m * 128:(m + 1) * 128],
                                 rhs=hT[2 + k][:, :], start=False, stop=(k == 1))
            for k in range(2):
                nc.tensor.matmul(P4[:, 2 + m, :],
                                 lhsT=Wi_bf[:, k, m * 128:(m + 1) * 128],
                                 rhs=hT[k][:, :], start=(k == 0), stop=False)
            for k in range(2):
                nc.tensor.matmul(P4[:, 2 + m, :],
                                 lhsT=Wr_bf[:, k, m * 128:(m + 1) * 128],
                                 rhs=hT[2 + k][:, :], start=False, stop=(k == 1))
        outT = rec.tile([128, 4, B], f32, tag="outT")
        for m in range(2):
            nc.scalar.activation(outT[:, m, :], P4[:, m, :], Tanh,
                                 bias=br_t[:, m:m + 1])
            nc.scalar.activation(outT[:, 2 + m, :], P4[:, 2 + m, :], Tanh,
                                 bias=bi_t[:, m:m + 1])
        out_sb = rec.tile([128, 512], f32, tag="out_sb")
        for j in range(4):
            pt = ptr.tile([128, 128], f32, tag="tr")
            nc.tensor.transpose(pt[0:B, 0:128], outT[:, j, :], ident[:, :])
            nc.any.tensor_copy(out_sb[0:B, j * 128:(j + 1) * 128], pt[0:B, 0:128])
        nc.sync.dma_start(outs["out"][:, :], out_sb[0:B, :])


_cached_nc = None


def _get_program():
    global _cached_nc
    if _cached_nc is None:
        nc = bacc.Bacc("TRN2", target_bir_lowering=False, debug=False)
        ins = {}
        for name, shape in SHAPES.items():
            ins[name] = nc.dram_tensor(name, list(shape), dt.float32,
                                       kind="ExternalInput")[...]
        out = nc.dram_tensor("out", [B, 512], dt.float32, kind="ExternalOutput")
        with tile.TileContext(nc) as tc:
            build(tc, {"out": out[...]}, ins)
        nc.finalize()  # bacc legalization (wait splitting, reg alloc, DCE)
        _cached_nc = nc
    return _cached_nc


def kernel(**inputs):
    nc = _get_program()
    in_maps = []
    for i in range(NCORES):
        m = {}
        for name in SHAPES:
            arr = np.ascontiguousarray(inputs[name], dtype=np.float32)
            if name in PER_BATCH:
                arr = np.ascontiguousarray(arr[i * B:(i + 1) * B])
            m[name] = arr
        in_maps.append(m)
    res = run_bass_kernel_spmd(nc, in_maps, list(range(NCORES)))
    return np.concatenate([res.results[i]["out"] for i in range(NCORES)], axis=0)


if __name__ == "__main__":
    import reference  # noqa: F401  (only for a local smoke run)
    inp = {k: np.asarray(v) for k, v in reference.setup_inputs().items()}
    out = kernel(**inp)
    print(out.shape, out.dtype)


# revision 13
# speedup vs baseline: 1.2624x; 1.0097x over previous
"""Trainium2 Bass kernel for nn_Critic (MLP preamble + 127-step LSTM + complex head).

Sharding: pure data-parallel over batch. B=256 -> 8 cores x 32. All weights
replicated; no collectives. Each core returns its [32, 512] output slice and
the host concatenates.

On-chip layout is "transposed" (feature-on-partition) end to end:
    h^T, c^T : [128 (hid%128), 4 (hid//128), 32 (batch)]
    z^T      : per-gate PSUM banks [128 (gate%128), 4 (gate//128), 32 (batch)]
so elementwise ops use all 128 lanes and h^T feeds the next step's matmul
(rhs) without any per-step transpose. All matmuls are bf16 inputs with fp32
PSUM accumulation (measured rel-L2 vs fp32 reference ~4e-3).

x@Wx + b_lstm is hoisted out of the scan. It is computed in 16-step chunks,
and each chunk's work (seq transpose + precompute matmuls) is WOVEN into the
recurrence steps of the previous chunk, so only chunk 0 runs up front: the
Tensor engine's per-engine FIFO means anything emitted before the recurrence
would fully serialize with it. The weaving also fills the PE idle holes in
each step's elementwise tail.

Recurrence step: X[t] is injected into the per-gate PSUM banks by the PE
itself (identity-stationary matmul, no h dependency), the k=0 contraction
level runs on h block 0 only (released first by a split h-mul), then each
gate's k=1..3 batch completes early so its activation overlaps later gates'
matmuls. Only sigmoid(o) -> h = o*tanh(c) trails the matmul burst.
"""

import numpy as np

import concourse.bass as bass  # noqa: F401
import concourse.mybir as mybir
from concourse import bacc, tile
from concourse.bass_utils import run_bass_kernel_spmd

dt = mybir.dt
AF = mybir.ActivationFunctionType

B = 32          # batch per core
NCORES = 8
T = 127         # scan steps (63 history + 64 action)
G = 2048        # gate dim
NM = G // 128   # 16 gate tiles
KH = 4          # hidden chunks of 128
KX = 2          # input chunks of 128
TC = 16         # time steps per pipeline chunk
# gate-tile order inside PSUM/XT blocks: [f, i, g, o] (original m-tile ids)
MPOS = [4, 5, 6, 7, 0, 1, 2, 3, 8, 9, 10, 11, 12, 13, 14, 15]

PER_BATCH = ("motion_state", "robot_state", "osc_state", "action", "osc", "history")
SHAPES = {
    "motion_state": (B, 128), "robot_state": (B, 64), "osc_state": (B, 128),
    "action": (B, 64, 256), "osc": (B, 128), "history": (B, 64, 256),
    "W_ros": (128, 256), "b_ros": (256,), "W_ios": (128, 256), "b_ios": (256,),
    "W_cos": (512, 256), "b_cos": (256,), "W_ms": (128, 256), "b_ms": (256,),
    "W_rs": (64, 256), "b_rs": (256,), "W_c": (768, 512), "b_c": (512,),
    "Wx": (256, 2048), "Wh": (512, 2048), "b_lstm": (2048,),
    "Wr_out": (256, 256), "Wi_out": (256, 256), "br_out": (256,), "bi_out": (256,),
}


def _bias_t(nc, pool, src_ap, n):
    """DMA a [n*128] bias vector into a [128, n] tile (partition = dim%128)."""
    t = pool.tile([128, n], dt.float32, name=f"bias_{src_ap.tensor.name}")
    nc.sync.dma_start(t[:, :], src_ap.rearrange("(j p) -> p j", p=128))
    return t


def build(tc, outs, ins, n_steps=T):
    nc = tc.nc
    f32, bf16 = dt.float32, dt.bfloat16
    Sig, Tanh = AF.Sigmoid, AF.Tanh
    nchunks = (n_steps + TC - 1) // TC

    import contextlib
    ctx = contextlib.ExitStack()
    with ctx:
        # ---------------- pools ----------------
        const = ctx.enter_context(tc.tile_pool(name="const", bufs=1))
        rec = ctx.enter_context(tc.tile_pool(name="rec", bufs=2))
        seqload = ctx.enter_context(tc.tile_pool(name="seqload", bufs=4))
        # PSUM: tr (transposes) + cp (preamble/precompute) + 4 gate banks = 8
        ptr = ctx.enter_context(tc.tile_pool(name="ptr", bufs=2, space="PSUM"))
        pz = ctx.enter_context(tc.tile_pool(name="pz", bufs=1, space="PSUM"))

        # identity via iota(j - p) == 0 — avoids affine_select's register
        # fill, which walrus fails to allocate on this compile path
        ident = const.tile([128, 128], f32, name="ident")
        iota_t = const.tile([128, 128], dt.int32, name="iota_t")
        nc.gpsimd.iota(iota_t[:, :], pattern=[[1, 128]], base=0,
                       channel_multiplier=-1)
        nc.vector.tensor_scalar(ident[:, :], iota_t[:, :], 0, None,
                                mybir.AluOpType.is_equal)
        ident_bf = const.tile([128, 128], bf16, name="ident_bf")
        nc.vector.tensor_copy(ident_bf[:, :], ident[:, :])

        # persistent weights / state
        Wh_bf = const.tile([128, KH, G], bf16, name="Wh_bf")
        Wx_bf = const.tile([128, KX, G], bf16, name="Wx_bf")
        Wr_bf = const.tile([128, 2, 256], bf16, name="Wr_bf")
        Wi_bf = const.tile([128, 2, 256], bf16, name="Wi_bf")
        Wineg_bf = const.tile([128, 2, 256], bf16, name="Wineg_bf")
        XTc = [const.tile([128, NM, min(TC, n_steps - j * TC), B], bf16,
                          name=f"XT{j}") for j in range(nchunks)]
        xTc = [const.tile([128, KX, min(TC, n_steps - j * TC) * B], bf16,
                          name=f"xT{j}") for j in range(nchunks)]
        c_st = const.tile([128, KH, B], f32, name="c_st")

        b_lstm_t = _bias_t(nc, const, ins["b_lstm"], NM)
        b_ros_t = _bias_t(nc, const, ins["b_ros"], 2)
        b_ios_t = _bias_t(nc, const, ins["b_ios"], 2)
        b_cos_t = _bias_t(nc, const, ins["b_cos"], 2)
        b_ms_t = _bias_t(nc, const, ins["b_ms"], 2)
        b_rs_t = _bias_t(nc, const, ins["b_rs"], 2)
        b_c_t = _bias_t(nc, const, ins["b_c"], 4)
        br_t = _bias_t(nc, const, ins["br_out"], 2)
        bi_t = _bias_t(nc, const, ins["bi_out"], 2)

        # ---- chunk-work emitters (phase B: transpose, phase C: precompute) --
        hist, act = ins["history"], ins["action"]

        def emit_B_pack(ch, pk):
            tcnt = min(TC, n_steps - ch * TC)
            t0 = ch * TC + 4 * pk
            nt = min(4, ch * TC + tcnt - t0)
            st = seqload.tile([128, 256], f32, tag="seq", name=f"seq{ch}_{pk}")
            tcur = t0
            while tcur < t0 + nt:
                if tcur < 63:
                    cnt = min(63 - tcur, t0 + nt - tcur)
                    src = hist[:, tcur:tcur + cnt, :]
                else:
                    cnt = t0 + nt - tcur
                    src = act[:, tcur - 63:tcur - 63 + cnt, :]
                row0 = (tcur - t0) * B
                nc.sync.dma_start(st[row0:row0 + cnt * B, :],
                                  src.rearrange("b t f -> t b f"))
                tcur += cnt
            for fc in range(KX):
                pt = ptr.tile([128, 128], f32, tag="tr", name=f"tr{ch}_{pk}_{fc}")
                nc.tensor.transpose(pt[:, 0:nt * B],
                                    st[0:nt * B, fc * 128:(fc + 1) * 128],
                                    ident[0:nt * B, 0:nt * B])
                nc.any.tensor_copy(xTc[ch][:, fc, 4 * pk * B:(4 * pk + nt) * B],
                                   pt[:, 0:nt * B])

        def emit_C_pos(ch, pos):
            tcnt = min(TC, n_steps - ch * TC)
            cols = tcnt * B
            m = MPOS[pos]
            P = ptr.tile([128, 512], f32, tag="cp", name=f"cp{ch}_{pos}")
            for k in range(KX):
                nc.tensor.matmul(P[:, 0:cols],
                                 lhsT=Wx_bf[:, k, m * 128:(m + 1) * 128],
                                 rhs=xTc[ch][:, k, 0:cols],
                                 start=(k == 0), stop=(k == KX - 1))
            XTm = XTc[ch][:, pos, :, :].rearrange("p t b -> p (t b)")
            if pos % 2 == 0:
                nc.scalar.activation(XTm[:, 0:cols], P[:, 0:cols], AF.Identity,
                                     bias=b_lstm_t[:, m:m + 1])
            else:
                nc.vector.tensor_scalar_add(XTm[:, 0:cols], P[:, 0:cols],
                                            b_lstm_t[:, m:m + 1])

        def chunk_thunks(ch):
            tcnt = min(TC, n_steps - ch * TC)
            th = [(emit_B_pack, ch, pk) for pk in range((tcnt + 3) // 4)]
            th += [(emit_C_pos, ch, pos) for pos in range(NM)]
            return th

        # ============ phase A: big weights -> bf16 sbuf ============
        with tc.tile_pool(name="wload", bufs=3) as wload:
            for k in range(KH):
                for h2 in range(2):
                    wtmp = wload.tile([128, 1024], f32, tag="wl")
                    nc.sync.dma_start(
                        wtmp[:, :], ins["Wh"][k * 128:(k + 1) * 128,
                                              h2 * 1024:(h2 + 1) * 1024])
                    nc.any.tensor_copy(
                        Wh_bf[:, k, h2 * 1024:(h2 + 1) * 1024], wtmp[:, :])
            for k in range(KX):
                for h2 in range(2):
                    wtmp = wload.tile([128, 1024], f32, tag="wl")
                    nc.sync.dma_start(
                        wtmp[:, :], ins["Wx"][k * 128:(k + 1) * 128,
                                              h2 * 1024:(h2 + 1) * 1024])
                    nc.any.tensor_copy(
                        Wx_bf[:, k, h2 * 1024:(h2 + 1) * 1024], wtmp[:, :])
            for k in range(2):
                wtmp = wload.tile([128, 1024], f32, tag="wl")
                nc.sync.dma_start(wtmp[:, 0:256],
                                  ins["Wr_out"][k * 128:(k + 1) * 128, :])
                nc.any.tensor_copy(Wr_bf[:, k, :], wtmp[:, 0:256])
                wtmp = wload.tile([128, 1024], f32, tag="wl")
                nc.sync.dma_start(wtmp[:, 0:256],
                                  ins["Wi_out"][k * 128:(k + 1) * 128, :])
                nc.any.tensor_copy(Wi_bf[:, k, :], wtmp[:, 0:256])
                nc.scalar.mul(Wineg_bf[:, k, :], wtmp[:, 0:256], -1.0)

        # ============ phase D: preamble MLP -> h0 = c0 = state^T ============
        with tc.tile_pool(name="dpool", bufs=1) as dpool, \
             tc.tile_pool(name="dload", bufs=2) as dload:

            def _load_w(name, kparts, n):
                wt = dpool.tile([128, kparts, n], bf16, name=f"{name}_bf")
                for k in range(kparts):
                    wtmp = dload.tile([128, 512], f32, tag="dl")
                    nc.sync.dma_start(wtmp[:, 0:n],
                                      ins[name][k * 128:(k + 1) * 128, :])
                    nc.any.tensor_copy(wt[:, k, :], wtmp[:, 0:n])
                return wt

            Wros_bf = _load_w("W_ros", 1, 256)
            Wios_bf = _load_w("W_ios", 1, 256)
            Wms_bf = _load_w("W_ms", 1, 256)
            Wcos_bf = _load_w("W_cos", 4, 256)
            Wc_bf = _load_w("W_c", 6, 512)
            Wrs_bf = dpool.tile([128, 1, 256], bf16, name="W_rs_bf")
            wtmp = dload.tile([128, 512], f32, tag="dl")
            nc.sync.dma_start(wtmp[0:64, 0:256], ins["W_rs"][:, :])
            nc.any.tensor_copy(Wrs_bf[0:64, 0, :], wtmp[0:64, 0:256])

            def _tr_in(name, rows):
                st = dload.tile([128, 512], f32, tag="dl")
                nc.sync.dma_start(st[0:B, 0:rows], ins[name][:, :])
                pt = ptr.tile([128, 128], f32, tag="tr")
                nc.tensor.transpose(pt[0:rows, 0:B], st[0:B, 0:rows],
                                    ident[0:B, 0:B])
                return pt

            p_mo = _tr_in("motion_state", 128)
            moT = dpool.tile([128, B], bf16, name="moT")
            nc.any.tensor_copy(moT[:, :], p_mo[:, 0:B])

            p_ro = _tr_in("robot_state", 64)
            roT = dpool.tile([128, B], bf16, name="roT")
            nc.any.tensor_copy(roT[0:64, :], p_ro[0:64, 0:B])

            reT = dpool.tile([128, B], bf16, name="reT")
            imT = dpool.tile([128, B], bf16, name="imT")
            p_os = _tr_in("osc_state", 128)
            nc.any.tensor_copy(reT[0:64, :], p_os[0:64, 0:B])
            nc.any.tensor_copy(imT[0:64, :], p_os[64:128, 0:B])
            p_oc = _tr_in("osc", 128)
            nc.any.tensor_copy(reT[64:128, :], p_oc[0:64, 0:B])
            nc.any.tensor_copy(imT[64:128, :], p_oc[64:128, 0:B])

            # stage 1: real_o / imag_o
            P1 = ptr.tile([128, 512], f32, tag="cp")
            for m in range(2):
                nc.tensor.matmul(P1[:, m * B:(m + 1) * B],
                                 lhsT=Wros_bf[:, 0, m * 128:(m + 1) * 128],
                                 rhs=reT[:, :], start=True, stop=True)
            for m in range(2):
                nc.tensor.matmul(P1[:, (2 + m) * B:(3 + m) * B],
                                 lhsT=Wios_bf[:, 0, m * 128:(m + 1) * 128],
                                 rhs=imT[:, :], start=True, stop=True)
            ro_bf = dpool.tile([128, 2, B], bf16, name="ro_bf")
            io_bf = dpool.tile([128, 2, B], bf16, name="io_bf")
            for m in range(2):
                nc.scalar.activation(ro_bf[:, m, :], P1[:, m * B:(m + 1) * B],
                                     Tanh, bias=b_ros_t[:, m:m + 1])
                nc.scalar.activation(io_bf[:, m, :],
                                     P1[:, (2 + m) * B:(3 + m) * B],
                                     Tanh, bias=b_ios_t[:, m:m + 1])

            # stage 2: ms, rs, osc_s
            P2 = ptr.tile([128, 512], f32, tag="cp")
            for m in range(2):
                nc.tensor.matmul(P2[:, m * B:(m + 1) * B],
                                 lhsT=Wms_bf[:, 0, m * 128:(m + 1) * 128],
                                 rhs=moT[:, :], start=True, stop=True)
            for m in range(2):
                nc.tensor.matmul(P2[:, (2 + m) * B:(3 + m) * B],
                                 lhsT=Wrs_bf[0:64, 0, m * 128:(m + 1) * 128],
                                 rhs=roT[0:64, :], start=True, stop=True)
            cos_chunks = [ro_bf[:, 0, :], ro_bf[:, 1, :],
                          io_bf[:, 0, :], io_bf[:, 1, :]]
            for m in range(2):
                for k in range(4):
                    nc.tensor.matmul(P2[:, (4 + m) * B:(5 + m) * B],
                                     lhsT=Wcos_bf[:, k, m * 128:(m + 1) * 128],
                                     rhs=cos_chunks[k],
                                     start=(k == 0), stop=(k == 3))
            ms_bf = dpool.tile([128, 2, B], bf16, name="ms_bf")
            rs_bf = dpool.tile([128, 2, B], bf16, name="rs_bf")
            os_bf = dpool.tile([128, 2, B], bf16, name="os_bf")
            for m in range(2):
                nc.scalar.activation(ms_bf[:, m, :], P2[:, m * B:(m + 1) * B],
                                     Tanh, bias=b_ms_t[:, m:m + 1])
                nc.scalar.activation(rs_bf[:, m, :],
                                     P2[:, (2 + m) * B:(3 + m) * B],
                                     Tanh, bias=b_rs_t[:, m:m + 1])
                nc.scalar.activation(os_bf[:, m, :],
                                     P2[:, (4 + m) * B:(5 + m) * B],
                                     Tanh, bias=b_cos_t[:, m:m + 1])

            # stage 3: state = tanh([ms rs osc_s] @ W_c + b_c) -> h0 = c0
            P3 = ptr.tile([128, 512], f32, tag="cp")
            st_chunks = [ms_bf[:, 0, :], ms_bf[:, 1, :], rs_bf[:, 0, :],
                         rs_bf[:, 1, :], os_bf[:, 0, :], os_bf[:, 1, :]]
            for m in range(KH):
                for k in range(6):
                    nc.tensor.matmul(P3[:, m * B:(m + 1) * B],
                                     lhsT=Wc_bf[:, k, m * 128:(m + 1) * 128],
                                     rhs=st_chunks[k],
                                     start=(k == 0), stop=(k == 5))
            hTb = [rec.tile([128, B], bf16, tag=f"h{k}", bufs=3, name=f"h0_{k}")
                   for k in range(KH)]
            for m in range(KH):
                nc.scalar.activation(c_st[:, m, :], P3[:, m * B:(m + 1) * B],
                                     Tanh, bias=b_c_t[:, m:m + 1])
            for k in range(KH):
                nc.vector.tensor_copy(hTb[k][:, :], c_st[:, k, :])
            hT = hTb

        # ============ chunk 0 up front ============
        for fn, ch, arg in chunk_thunks(0):
            fn(ch, arg)

        # ============ phase E: LSTM recurrence, weaving chunk ch+1 ==========
        for t in range(n_steps):
            ch, tl = t // TC, t % TC
            Zf = pz.tile([128, KH, B], f32, tag="zf")
            Zi = pz.tile([128, KH, B], f32, tag="zi")
            Zg = pz.tile([128, KH, B], f32, tag="zg")
            Zo = pz.tile([128, KH, B], f32, tag="zo")
            Zs = [Zf, Zi, Zg, Zo]
            gf = rec.tile([128, KH, B], f32, tag="gf")
            gi = rec.tile([128, KH, B], f32, tag="gi")
            gg = rec.tile([128, KH, B], f32, tag="gg")
            go = rec.tile([128, KH, B], f32, tag="go")
            tmp = rec.tile([128, KH, B], f32, tag="tmp")
            tanh_c = rec.tile([128, KH, B], f32, tag="tanhc")
            hTb_n = [rec.tile([128, B], bf16, tag=f"h{k}", bufs=3,
                              name=f"h{t}_{k}") for k in range(KH)]

            # X[t] injected into PSUM by the PE itself (identity stationary):
            # no h dependency, so these run during the previous step's tail,
            # and the activations can read the finished bank directly.
            for gidx in range(4):
                nc.tensor.matmul(Zs[gidx][:, :, :], lhsT=ident_bf[:, :],
                                 rhs=XTc[ch][:, gidx * 4:gidx * 4 + 4, tl, :],
                                 start=True, stop=False, skip_group_check=True)
            # k=0 level for all 16 gate tiles: only needs h block 0, which the
            # split h-mul below releases first
            for gidx in range(4):
                for blk in range(4):
                    m = MPOS[gidx * 4 + blk]
                    nc.tensor.matmul(Zs[gidx][:, blk, :],
                                     lhsT=Wh_bf[:, 0, m * 128:(m + 1) * 128],
                                     rhs=hT[0][:, :], start=False, stop=False,
                                     skip_group_check=True)
            # per-gate k=1..3 batches; each gate's bank completes early so its
            # activation overlaps the later gates' matmuls
            for gidx in range(4):
                Z = Zs[gidx]
                for blk in range(4):
                    m = MPOS[gidx * 4 + blk]
                    for k in range(1, KH):
                        nc.tensor.matmul(
                            Z[:, blk, :],
                            lhsT=Wh_bf[:, k, m * 128:(m + 1) * 128],
                            rhs=hT[k][:, :], start=False,
                            stop=(k == KH - 1 and blk == 3),
                            skip_group_check=True)
                if gidx == 0:
                    nc.scalar.activation(gf[:, :, :], Z[:, :, :], Sig)
                elif gidx == 1:
                    nc.scalar.activation(gi[:, :, :], Z[:, :, :], Sig)
                    nc.vector.tensor_mul(c_st[:, :, :], gf[:, :, :],
                                         c_st[:, :, :])
                elif gidx == 2:
                    nc.scalar.activation(gg[:, :, :], Z[:, :, :], Tanh)
                    nc.vector.tensor_mul(tmp[:, :, :], gi[:, :, :], gg[:, :, :])
                    nc.vector.tensor_add(c_st[:, :, :], c_st[:, :, :],
                                         tmp[:, :, :])
                else:
                    nc.scalar.activation(go[:, :, :], Z[:, :, :], Sig)
                    nc.scalar.activation(tanh_c[:, :, :], c_st[:, :, :], Tanh)
                    for k in range(KH):
                        nc.vector.tensor_mul(hTb_n[k][:, :], go[:, k, :],
                                             tanh_c[:, k, :])
            hT = hTb_n

            # weave next chunk's transpose+precompute into this step's slack
            if ch + 1 < nchunks and tl < 12:
                th = chunk_thunks(ch + 1)
                lo = (len(th) * tl + 11) // 12
                hi = (len(th) * (tl + 1) + 11) // 12
                for fn, c2, arg in th[lo:hi]:
                    fn(c2, arg)

        # ============ phase F: complex dense head + output transpose ========
        P4 = pz.tile([128, KH, B], f32, tag="zf")
        for m in range(2):
            for k in range(2):
                nc.tensor.matmul(P4[:, m, :],
                                 lhsT=Wr_bf[:, k, m * 128:(m + 1) * 128],
                                 rhs=hT[k][:, :], start=(k == 0), stop=False)
            for k in range(2):
                nc.tensor.matmul(P4[:, m, :],
                                 lhsT=Wineg_bf[:, k, m * 128:(m + 1) * 128],
                                 rhs=hT[2 + k][:, :], start=False, stop=(k == 1))
            for k in range(2):
                nc.tensor.matmul(P4[:, 2 + m, :],
                                 lhsT=Wi_bf[:, k, m * 128:(m + 1) * 128],
                                 rhs=hT[k][:, :], start=(k == 0), stop=False)
            for k in range(2):
                nc.tensor.matmul(P4[:, 2 + m, :],
                                 lhsT=Wr_bf[:, k, m * 128:(m + 1) * 128],
                                 rhs=hT[2 + k][:, :], start=False, stop=(k == 1))
        outT = rec.tile([128, 4, B], f32, tag="outT")
        for m in range(2):
            nc.scalar.activation(outT[:, m, :], P4[:, m, :], Tanh,
                                 bias=br_t[:, m:m + 1])
            nc.scalar.activation(outT[:, 2 + m, :], P4[:, 2 + m, :], Tanh,
                                 bias=bi_t[:, m:m + 1])
        out_sb = rec.tile([128, 512], f32, tag="out_sb")
        for j in range(4):
            pt = ptr.tile([128, 128], f32, tag="tr")
            nc.tensor.transpose(pt[0:B, 0:128], outT[:, j, :], ident[:, :])
            nc.any.tensor_copy(out_sb[0:B, j * 128:(j + 1) * 128], pt[0:B, 0:128])
        nc.sync.dma_start(outs["out"][:, :], out_sb[0:B, :])


_cached_nc = None


def _get_program():
    global _cached_nc
    if _cached_nc is None:
        nc = bacc.Bacc("TRN2", target_bir_lowering=False, debug=False)
        ins = {}
        for name, shape in SHAPES.items():
            ins[name] = nc.dram_tensor(name, list(shape), dt.float32,
                                       kind="ExternalInput")[...]
        out = nc.dram_tensor("out", [B, 512], dt.float32, kind="ExternalOutput")
        with tile.TileContext(nc) as tc:
            build(tc, {"out": out[...]}, ins)
        nc.finalize()  # bacc legalization (wait splitting, reg alloc, DCE)
        _cached_nc = nc
    return _cached_nc


def kernel(**inputs):
    nc = _get_program()
    in_maps = []
    for i in range(NCORES):
        m = {}
        for name in SHAPES:
            arr = np.ascontiguousarray(inputs[name], dtype=np.float32)
            if name in PER_BATCH:
                arr = np.ascontiguousarray(arr[i * B:(i + 1) * B])
            m[name] = arr
        in_maps.append(m)
    res = run_bass_kernel_spmd(nc, in_maps, list(range(NCORES)))
    return np.concatenate([res.results[i]["out"] for i in range(NCORES)], axis=0)


if __name__ == "__main__":
    import reference  # noqa: F401  (only for a local smoke run)
    inp = {k: np.asarray(v) for k, v in reference.setup_inputs().items()}
    out = kernel(**inp)
    print(out.shape, out.dtype)


# revision 15
# speedup vs baseline: 1.2678x; 1.0043x over previous
"""Trainium2 Bass kernel for nn_Critic (MLP preamble + 127-step LSTM + complex head).

Sharding: pure data-parallel over batch. B=256 -> 8 cores x 32. All weights
replicated; no collectives. Each core returns its [32, 512] output slice and
the host concatenates.

On-chip layout is "transposed" (feature-on-partition) end to end:
    h^T, c^T : [128 (hid%128), 4 (hid//128), 32 (batch)]
    z^T      : per-gate PSUM banks [128 (gate%128), 4 (gate//128), 32 (batch)]
so elementwise ops use all 128 lanes and h^T feeds the next step's matmul
(rhs) without any per-step transpose. All matmuls are bf16 inputs with fp32
PSUM accumulation (measured rel-L2 vs fp32 reference ~4e-3).

x@Wx + b_lstm is hoisted out of the scan. It is computed in 16-step chunks,
and each chunk's work (seq transpose + precompute matmuls) is WOVEN into the
recurrence steps of the previous chunk, so only chunk 0 runs up front: the
Tensor engine's per-engine FIFO means anything emitted before the recurrence
would fully serialize with it. The weaving also fills the PE idle holes in
each step's elementwise tail.

Recurrence step: X[t] is injected into the per-gate PSUM banks by the PE
itself (identity-stationary matmul, no h dependency), the k=0 contraction
level runs on h block 0 only (released first by a split h-mul), then each
gate's k=1..3 batch completes early so its activation overlaps later gates'
matmuls. Only sigmoid(o) -> h = o*tanh(c) trails the matmul burst.
"""

import numpy as np

import concourse.bass as bass  # noqa: F401
import concourse.mybir as mybir
from concourse import bacc, tile
from concourse.bass_utils import run_bass_kernel_spmd

dt = mybir.dt
AF = mybir.ActivationFunctionType

B = 32          # batch per core
NCORES = 8
T = 127         # scan steps (63 history + 64 action)
G = 2048        # gate dim
NM = G // 128   # 16 gate tiles
KH = 4          # hidden chunks of 128
KX = 2          # input chunks of 128
TC = 16         # time steps per pipeline chunk
# gate-tile order inside PSUM/XT blocks: [f, i, g, o] (original m-tile ids)
MPOS = [4, 5, 6, 7, 0, 1, 2, 3, 8, 9, 10, 11, 12, 13, 14, 15]

PER_BATCH = ("motion_state", "robot_state", "osc_state", "action", "osc", "history")
SHAPES = {
    "motion_state": (B, 128), "robot_state": (B, 64), "osc_state": (B, 128),
    "action": (B, 64, 256), "osc": (B, 128), "history": (B, 64, 256),
    "W_ros": (128, 256), "b_ros": (256,), "W_ios": (128, 256), "b_ios": (256,),
    "W_cos": (512, 256), "b_cos": (256,), "W_ms": (128, 256), "b_ms": (256,),
    "W_rs": (64, 256), "b_rs": (256,), "W_c": (768, 512), "b_c": (512,),
    "Wx": (256, 2048), "Wh": (512, 2048), "b_lstm": (2048,),
    "Wr_out": (256, 256), "Wi_out": (256, 256), "br_out": (256,), "bi_out": (256,),
}


def _bias_t(nc, pool, src_ap, n):
    """DMA a [n*128] bias vector into a [128, n] tile (partition = dim%128)."""
    t = pool.tile([128, n], dt.float32, name=f"bias_{src_ap.tensor.name}")
    nc.sync.dma_start(t[:, :], src_ap.rearrange("(j p) -> p j", p=128))
    return t


def build(tc, outs, ins, n_steps=T):
    nc = tc.nc
    f32, bf16 = dt.float32, dt.bfloat16
    Sig, Tanh = AF.Sigmoid, AF.Tanh
    nchunks = (n_steps + TC - 1) // TC

    import contextlib
    ctx = contextlib.ExitStack()
    with ctx:
        # ---------------- pools ----------------
        const = ctx.enter_context(tc.tile_pool(name="const", bufs=1))
        rec = ctx.enter_context(tc.tile_pool(name="rec", bufs=2))
        seqload = ctx.enter_context(tc.tile_pool(name="seqload", bufs=4))
        # PSUM: tr (transposes) + cp (preamble/precompute) + 4 gate banks = 8
        ptr = ctx.enter_context(tc.tile_pool(name="ptr", bufs=2, space="PSUM"))
        pz = ctx.enter_context(tc.tile_pool(name="pz", bufs=1, space="PSUM"))

        # identity via iota(j - p) == 0 — avoids affine_select's register
        # fill, which walrus fails to allocate on this compile path
        ident = const.tile([128, 128], f32, name="ident")
        iota_t = const.tile([128, 128], dt.int32, name="iota_t")
        nc.gpsimd.iota(iota_t[:, :], pattern=[[1, 128]], base=0,
                       channel_multiplier=-1)
        nc.vector.tensor_scalar(ident[:, :], iota_t[:, :], 0, None,
                                mybir.AluOpType.is_equal)
        ident_bf = const.tile([128, 128], bf16, name="ident_bf")
        nc.vector.tensor_copy(ident_bf[:, :], ident[:, :])

        # persistent weights / state
        Wh_bf = const.tile([128, KH, G], bf16, name="Wh_bf")
        Wx_bf = const.tile([128, KX, G], bf16, name="Wx_bf")
        Wr_bf = const.tile([128, 2, 256], bf16, name="Wr_bf")
        Wi_bf = const.tile([128, 2, 256], bf16, name="Wi_bf")
        Wineg_bf = const.tile([128, 2, 256], bf16, name="Wineg_bf")
        XTc = [const.tile([128, NM, min(TC, n_steps - j * TC), B], bf16,
                          name=f"XT{j}") for j in range(nchunks)]
        xTc = [const.tile([128, KX, min(TC, n_steps - j * TC) * B], bf16,
                          name=f"xT{j}") for j in range(nchunks)]
        c_st = const.tile([128, KH, B], f32, name="c_st")

        b_lstm_t = _bias_t(nc, const, ins["b_lstm"], NM)
        b_ros_t = _bias_t(nc, const, ins["b_ros"], 2)
        b_ios_t = _bias_t(nc, const, ins["b_ios"], 2)
        b_cos_t = _bias_t(nc, const, ins["b_cos"], 2)
        b_ms_t = _bias_t(nc, const, ins["b_ms"], 2)
        b_rs_t = _bias_t(nc, const, ins["b_rs"], 2)
        b_c_t = _bias_t(nc, const, ins["b_c"], 4)
        br_t = _bias_t(nc, const, ins["br_out"], 2)
        bi_t = _bias_t(nc, const, ins["bi_out"], 2)

        # ---- chunk-work emitters (phase B: transpose, phase C: precompute) --
        hist, act = ins["history"], ins["action"]

        def emit_B_pack(ch, pk):
            tcnt = min(TC, n_steps - ch * TC)
            t0 = ch * TC + 4 * pk
            nt = min(4, ch * TC + tcnt - t0)
            st = seqload.tile([128, 256], f32, tag="seq", name=f"seq{ch}_{pk}")
            tcur = t0
            while tcur < t0 + nt:
                if tcur < 63:
                    cnt = min(63 - tcur, t0 + nt - tcur)
                    src = hist[:, tcur:tcur + cnt, :]
                else:
                    cnt = t0 + nt - tcur
                    src = act[:, tcur - 63:tcur - 63 + cnt, :]
                row0 = (tcur - t0) * B
                nc.sync.dma_start(st[row0:row0 + cnt * B, :],
                                  src.rearrange("b t f -> t b f"))
                tcur += cnt
            for fc in range(KX):
                pt = ptr.tile([128, 128], f32, tag="tr", name=f"tr{ch}_{pk}_{fc}")
                nc.tensor.transpose(pt[:, 0:nt * B],
                                    st[0:nt * B, fc * 128:(fc + 1) * 128],
                                    ident[0:nt * B, 0:nt * B])
                nc.vector.tensor_copy(xTc[ch][:, fc, 4 * pk * B:(4 * pk + nt) * B],
                                      pt[:, 0:nt * B])

        def emit_C_pos(ch, pos):
            tcnt = min(TC, n_steps - ch * TC)
            cols = tcnt * B
            m = MPOS[pos]
            P = ptr.tile([128, 512], f32, tag="cp", name=f"cp{ch}_{pos}")
            for k in range(KX):
                nc.tensor.matmul(P[:, 0:cols],
                                 lhsT=Wx_bf[:, k, m * 128:(m + 1) * 128],
                                 rhs=xTc[ch][:, k, 0:cols],
                                 start=(k == 0), stop=(k == KX - 1))
            XTm = XTc[ch][:, pos, :, :].rearrange("p t b -> p (t b)")
            # DVE only: the Scalar engine is the recurrence's critical engine
            nc.vector.tensor_scalar_add(XTm[:, 0:cols], P[:, 0:cols],
                                        b_lstm_t[:, m:m + 1])

        def chunk_thunks(ch):
            tcnt = min(TC, n_steps - ch * TC)
            th = [(emit_B_pack, ch, pk) for pk in range((tcnt + 3) // 4)]
            th += [(emit_C_pos, ch, pos) for pos in range(NM)]
            return th

        # ============ phase A: big weights -> bf16 sbuf ============
        with tc.tile_pool(name="wload", bufs=3) as wload:
            for k in range(KX):
                for h2 in range(2):
                    wtmp = wload.tile([128, 1024], f32, tag="wl")
                    nc.sync.dma_start(
                        wtmp[:, :], ins["Wx"][k * 128:(k + 1) * 128,
                                              h2 * 1024:(h2 + 1) * 1024])
                    nc.any.tensor_copy(
                        Wx_bf[:, k, h2 * 1024:(h2 + 1) * 1024], wtmp[:, :])
            for k in range(KH):
                for h2 in range(2):
                    wtmp = wload.tile([128, 1024], f32, tag="wl")
                    nc.sync.dma_start(
                        wtmp[:, :], ins["Wh"][k * 128:(k + 1) * 128,
                                              h2 * 1024:(h2 + 1) * 1024])
                    nc.any.tensor_copy(
                        Wh_bf[:, k, h2 * 1024:(h2 + 1) * 1024], wtmp[:, :])
            for k in range(2):
                wtmp = wload.tile([128, 1024], f32, tag="wl")
                nc.sync.dma_start(wtmp[:, 0:256],
                                  ins["Wr_out"][k * 128:(k + 1) * 128, :])
                nc.any.tensor_copy(Wr_bf[:, k, :], wtmp[:, 0:256])
                wtmp = wload.tile([128, 1024], f32, tag="wl")
                nc.sync.dma_start(wtmp[:, 0:256],
                                  ins["Wi_out"][k * 128:(k + 1) * 128, :])
                nc.any.tensor_copy(Wi_bf[:, k, :], wtmp[:, 0:256])
                nc.scalar.mul(Wineg_bf[:, k, :], wtmp[:, 0:256], -1.0)

        # ============ phase D: preamble MLP -> h0 = c0 = state^T ============
        with tc.tile_pool(name="dpool", bufs=1) as dpool, \
             tc.tile_pool(name="dload", bufs=2) as dload:

            def _load_w(name, kparts, n):
                wt = dpool.tile([128, kparts, n], bf16, name=f"{name}_bf")
                for k in range(kparts):
                    wtmp = dload.tile([128, 512], f32, tag="dl")
                    nc.sync.dma_start(wtmp[:, 0:n],
                                      ins[name][k * 128:(k + 1) * 128, :])
                    nc.any.tensor_copy(wt[:, k, :], wtmp[:, 0:n])
                return wt

            Wros_bf = _load_w("W_ros", 1, 256)
            Wios_bf = _load_w("W_ios", 1, 256)
            Wms_bf = _load_w("W_ms", 1, 256)
            Wcos_bf = _load_w("W_cos", 4, 256)
            Wc_bf = _load_w("W_c", 6, 512)
            Wrs_bf = dpool.tile([128, 1, 256], bf16, name="W_rs_bf")
            wtmp = dload.tile([128, 512], f32, tag="dl")
            nc.sync.dma_start(wtmp[0:64, 0:256], ins["W_rs"][:, :])
            nc.any.tensor_copy(Wrs_bf[0:64, 0, :], wtmp[0:64, 0:256])

            def _tr_in(name, rows):
                st = dload.tile([128, 512], f32, tag="dl")
                nc.sync.dma_start(st[0:B, 0:rows], ins[name][:, :])
                pt = ptr.tile([128, 128], f32, tag="tr")
                nc.tensor.transpose(pt[0:rows, 0:B], st[0:B, 0:rows],
                                    ident[0:B, 0:B])
                return pt

            p_mo = _tr_in("motion_state", 128)
            moT = dpool.tile([128, B], bf16, name="moT")
            nc.any.tensor_copy(moT[:, :], p_mo[:, 0:B])

            p_ro = _tr_in("robot_state", 64)
            roT = dpool.tile([128, B], bf16, name="roT")
            nc.any.tensor_copy(roT[0:64, :], p_ro[0:64, 0:B])

            reT = dpool.tile([128, B], bf16, name="reT")
            imT = dpool.tile([128, B], bf16, name="imT")
            p_os = _tr_in("osc_state", 128)
            nc.any.tensor_copy(reT[0:64, :], p_os[0:64, 0:B])
            nc.any.tensor_copy(imT[0:64, :], p_os[64:128, 0:B])
            p_oc = _tr_in("osc", 128)
            nc.any.tensor_copy(reT[64:128, :], p_oc[0:64, 0:B])
            nc.any.tensor_copy(imT[64:128, :], p_oc[64:128, 0:B])

            # stage 1: real_o / imag_o
            P1 = ptr.tile([128, 512], f32, tag="cp")
            for m in range(2):
                nc.tensor.matmul(P1[:, m * B:(m + 1) * B],
                                 lhsT=Wros_bf[:, 0, m * 128:(m + 1) * 128],
                                 rhs=reT[:, :], start=True, stop=True)
            for m in range(2):
                nc.tensor.matmul(P1[:, (2 + m) * B:(3 + m) * B],
                                 lhsT=Wios_bf[:, 0, m * 128:(m + 1) * 128],
                                 rhs=imT[:, :], start=True, stop=True)
            ro_bf = dpool.tile([128, 2, B], bf16, name="ro_bf")
            io_bf = dpool.tile([128, 2, B], bf16, name="io_bf")
            for m in range(2):
                nc.scalar.activation(ro_bf[:, m, :], P1[:, m * B:(m + 1) * B],
                                     Tanh, bias=b_ros_t[:, m:m + 1])
                nc.scalar.activation(io_bf[:, m, :],
                                     P1[:, (2 + m) * B:(3 + m) * B],
                                     Tanh, bias=b_ios_t[:, m:m + 1])

            # stage 2: ms, rs, osc_s
            P2 = ptr.tile([128, 512], f32, tag="cp")
            for m in range(2):
                nc.tensor.matmul(P2[:, m * B:(m + 1) * B],
                                 lhsT=Wms_bf[:, 0, m * 128:(m + 1) * 128],
                                 rhs=moT[:, :], start=True, stop=True)
            for m in range(2):
                nc.tensor.matmul(P2[:, (2 + m) * B:(3 + m) * B],
                                 lhsT=Wrs_bf[0:64, 0, m * 128:(m + 1) * 128],
                                 rhs=roT[0:64, :], start=True, stop=True)
            cos_chunks = [ro_bf[:, 0, :], ro_bf[:, 1, :],
                          io_bf[:, 0, :], io_bf[:, 1, :]]
            for m in range(2):
                for k in range(4):
                    nc.tensor.matmul(P2[:, (4 + m) * B:(5 + m) * B],
                                     lhsT=Wcos_bf[:, k, m * 128:(m + 1) * 128],
                                     rhs=cos_chunks[k],
                                     start=(k == 0), stop=(k == 3))
            ms_bf = dpool.tile([128, 2, B], bf16, name="ms_bf")
            rs_bf = dpool.tile([128, 2, B], bf16, name="rs_bf")
            os_bf = dpool.tile([128, 2, B], bf16, name="os_bf")
            for m in range(2):
                nc.scalar.activation(ms_bf[:, m, :], P2[:, m * B:(m + 1) * B],
                                     Tanh, bias=b_ms_t[:, m:m + 1])
                nc.scalar.activation(rs_bf[:, m, :],
                                     P2[:, (2 + m) * B:(3 + m) * B],
                                     Tanh, bias=b_rs_t[:, m:m + 1])
                nc.scalar.activation(os_bf[:, m, :],
                                     P2[:, (4 + m) * B:(5 + m) * B],
                                     Tanh, bias=b_cos_t[:, m:m + 1])

            # stage 3: state = tanh([ms rs osc_s] @ W_c + b_c) -> h0 = c0
            P3 = ptr.tile([128, 512], f32, tag="cp")
            st_chunks = [ms_bf[:, 0, :], ms_bf[:, 1, :], rs_bf[:, 0, :],
                         rs_bf[:, 1, :], os_bf[:, 0, :], os_bf[:, 1, :]]
            for m in range(KH):
                for k in range(6):
                    nc.tensor.matmul(P3[:, m * B:(m + 1) * B],
                                     lhsT=Wc_bf[:, k, m * 128:(m + 1) * 128],
                                     rhs=st_chunks[k],
                                     start=(k == 0), stop=(k == 5))
            hTb = [rec.tile([128, B], bf16, tag=f"h{k}", bufs=3, name=f"h0_{k}")
                   for k in range(KH)]
            for m in range(KH):
                nc.scalar.activation(c_st[:, m, :], P3[:, m * B:(m + 1) * B],
                                     Tanh, bias=b_c_t[:, m:m + 1])
            for k in range(KH):
                nc.vector.tensor_copy(hTb[k][:, :], c_st[:, k, :])
            hT = hTb

        # ============ chunk 0 up front ============
        for fn, ch, arg in chunk_thunks(0):
            fn(ch, arg)

        # ============ phase E: LSTM recurrence, weaving chunk ch+1 ==========
        for t in range(n_steps):
            ch, tl = t // TC, t % TC
            Zfi = pz.tile([128, 2 * KH, B], f32, tag="zfi")   # f+i, one bank
            Zg = pz.tile([128, KH, B], f32, tag="zg")
            Zo = pz.tile([128, KH, B], f32, tag="zo")
            gfi = rec.tile([128, 2 * KH, B], f32, tag="gfi")
            gg = rec.tile([128, KH, B], f32, tag="gg")
            go = rec.tile([128, KH, B], f32, tag="go")
            tmp = rec.tile([128, KH, B], f32, tag="tmp")
            tanh_c = rec.tile([128, KH, B], f32, tag="tanhc")
            hTb_n = [rec.tile([128, B], bf16, tag=f"h{k}", bufs=3,
                              name=f"h{t}_{k}") for k in range(KH)]
            zdst = ([(Zfi, blk) for blk in range(8)]
                    + [(Zg, blk) for blk in range(4)]
                    + [(Zo, blk) for blk in range(4)])

            # X[t] injected into PSUM by the PE itself (identity stationary):
            # no h dependency, so these run during the previous step's tail,
            # and the activations can read the finished bank directly.
            nc.tensor.matmul(Zfi[:, :, :], lhsT=ident_bf[:, :],
                             rhs=XTc[ch][:, 0:8, tl, :],
                             start=True, stop=False, skip_group_check=True)
            nc.tensor.matmul(Zg[:, :, :], lhsT=ident_bf[:, :],
                             rhs=XTc[ch][:, 8:12, tl, :],
                             start=True, stop=False, skip_group_check=True)
            nc.tensor.matmul(Zo[:, :, :], lhsT=ident_bf[:, :],
                             rhs=XTc[ch][:, 12:16, tl, :],
                             start=True, stop=False, skip_group_check=True)
            # k=0 level for all 16 gate tiles: only needs h block 0, which the
            # split h-mul below releases first
            for pos in range(NM):
                Z, blk = zdst[pos]
                m = MPOS[pos]
                nc.tensor.matmul(Z[:, blk, :],
                                 lhsT=Wh_bf[:, 0, m * 128:(m + 1) * 128],
                                 rhs=hT[0][:, :], start=False, stop=False,
                                 skip_group_check=True)
            # k=1..3 batches per bank; each bank completes early so its
            # activation overlaps the later banks' matmuls
            for gidx, (Z, blks) in enumerate([(Zfi, 8), (Zg, 4), (Zo, 4)]):
                for blk in range(blks):
                    pos = blk if gidx == 0 else 8 + (gidx - 1) * 4 + blk
                    m = MPOS[pos]
                    for k in range(1, KH):
                        nc.tensor.matmul(
                            Z[:, blk, :],
                            lhsT=Wh_bf[:, k, m * 128:(m + 1) * 128],
                            rhs=hT[k][:, :], start=False,
                            stop=(k == KH - 1 and blk == blks - 1),
                            skip_group_check=True)
                if gidx == 0:
                    # merged sigmoid over f and i (one bank, one table pass)
                    nc.scalar.activation(gfi[:, :, :], Z[:, :, :], Sig)
                    nc.vector.tensor_mul(c_st[:, :, :], gfi[:, 0:4, :],
                                         c_st[:, :, :])
                elif gidx == 1:
                    nc.scalar.activation(gg[:, :, :], Z[:, :, :], Tanh)
                    nc.vector.tensor_mul(tmp[:, :, :], gfi[:, 4:8, :],
                                         gg[:, :, :])
                    nc.vector.tensor_add(c_st[:, :, :], c_st[:, :, :],
                                         tmp[:, :, :])
                else:
                    nc.scalar.activation(go[:, :, :], Z[:, :, :], Sig)
                    nc.scalar.activation(tanh_c[:, :, :], c_st[:, :, :], Tanh)
                    for k in range(KH):
                        nc.vector.tensor_mul(hTb_n[k][:, :], go[:, k, :],
                                             tanh_c[:, k, :])
            hT = hTb_n

            # weave next chunk's transpose+precompute into this step's slack
            if ch + 1 < nchunks and tl < 12:
                th = chunk_thunks(ch + 1)
                lo = (len(th) * tl + 11) // 12
                hi = (len(th) * (tl + 1) + 11) // 12
                for fn, c2, arg in th[lo:hi]:
                    fn(c2, arg)

        # ============ phase F: complex dense head + output transpose ========
        P4 = pz.tile([128, KH, B], f32, tag="zg")
        for m in range(2):
            for k in range(2):
                nc.tensor.matmul(P4[:, m, :],
                                 lhsT=Wr_bf[:, k, m * 128:(m + 1) * 128],
                                 rhs=hT[k][:, :], start=(k == 0), stop=False)
            for k in range(2):
                nc.tensor.matmul(P4[:, m, :],
                                 lhsT=Wineg_bf[:, k, m * 128:(m + 1) * 128],
                                 rhs=hT[2 + k][:, :], start=False, stop=(k == 1))
            for k in range(2):
                nc.tensor.matmul(P4[:, 2 + m, :],
                                 lhsT=Wi_bf[:, k, m * 128:(m + 1) * 128],
                                 rhs=hT[k][:, :], start=(k == 0), stop=False)
            for k in range(2):
                nc.tensor.matmul(P4[:, 2 + m, :],
                                 lhsT=Wr_bf[:, k, m * 128:(m + 1) * 128],
                                 rhs=hT[2 + k][:, :], start=False, stop=(k == 1))
        outT = rec.tile([128, 4, B], f32, tag="outT")
        for m in range(2):
            nc.scalar.activation(outT[:, m, :], P4[:, m, :], Tanh,
                                 bias=br_t[:, m:m + 1])
            nc.scalar.activation(outT[:, 2 + m, :], P4[:, 2 + m, :], Tanh,
                                 bias=bi_t[:, m:m + 1])
        out_sb = rec.tile([128, 512], f32, tag="out_sb")
        for j in range(4):
            pt = ptr.tile([128, 128], f32, tag="tr")
            nc.tensor.transpose(pt[0:B, 0:128], outT[:, j, :], ident[:, :])
            nc.any.tensor_copy(out_sb[0:B, j * 128:(j + 1) * 128], pt[0:B, 0:128])
        nc.sync.dma_start(outs["out"][:, :], out_sb[0:B, :])


_cached_nc = None


def _get_program():
    global _cached_nc
    if _cached_nc is None:
        nc = bacc.Bacc("TRN2", target_bir_lowering=False, debug=False)
        ins = {}
        for name, shape in SHAPES.items():
            ins[name] = nc.dram_tensor(name, list(shape), dt.float32,
                                       kind="ExternalInput")[...]
        out = nc.dram_tensor("out", [B, 512], dt.float32, kind="ExternalOutput")
        with tile.TileContext(nc) as tc:
            build(tc, {"out": out[...]}, ins)
        nc.finalize()  # bacc legalization (wait splitting, reg alloc, DCE)
        _cached_nc = nc
    return _cached_nc


def kernel(**inputs):
    nc = _get_program()
    in_maps = []
    for i in range(NCORES):
        m = {}
        for name in SHAPES:
            arr = np.ascontiguousarray(inputs[name], dtype=np.float32)
            if name in PER_BATCH:
                arr = np.ascontiguousarray(arr[i * B:(i + 1) * B])
            m[name] = arr
        in_maps.append(m)
    res = run_bass_kernel_spmd(nc, in_maps, list(range(NCORES)))
    return np.concatenate([res.results[i]["out"] for i in range(NCORES)], axis=0)


if __name__ == "__main__":
    import reference  # noqa: F401  (only for a local smoke run)
    inp = {k: np.asarray(v) for k, v in reference.setup_inputs().items()}
    out = kernel(**inp)
    print(out.shape, out.dtype)
